# revision 20
# baseline (speedup 1.0000x reference)
"""Trainium2 Bass kernel for nn_Block_Attention_3 (sparse_attention).

Contract: kernel(**inputs) takes FULL fp32 inputs (as in reference.setup_inputs())
and returns the FULL (4, 2304, 16, 16) fp32 output.

Strategy (zero-collective position sharding + fp8 weight streaming):
  The image is 16x16 = 4x4 grid of 4x4 patches. All cross-position coupling in
  the block stays within one (batch, patch-row) group, so the 16 units (b, i)
  shard cleanly across 8 cores, 2 units/core, with weights replicated.
  Replication is the whole cost: the kernel is DMA-stream-bound on the four
  2048x256 conv weight matrices. They are streamed as float8e3 (e3m4) with
  per-tensor scales, halving weight traffic vs bf16; x streams as e3m4 too.
  Every descale folds into host-prepared operands (R, pos copies, bias rows)
  or rides per-partition scale columns, so the compiled program is
  input-independent and exact up to the fp8 rounding itself.

Per-core pipeline (single Bass program, SPMD over 8 cores):
  - inference BN folded into conv weights/biases on host; out-BN scale into the
    V path; v-bias + k-bias ride the pos operands; bd/bq/beta enter as rank-1
    matmuls at the END of each PSUM accumulation group.
  - pixels are laid out patch-major: pix = u*64 + 16*jp + 4*ph + pw.
  - d,v convs accumulate in A-layout [pix, outch]; k,q convs in B-layout
    [outch, pix] so the scores matmul needs no transposes.
  - big tensors stream via SP-queue HWDGE (9 DMAs, sized so desc-gen stays
    ahead of the 360 B/ns transfer stream); small aux tensors go via Pool
    SWDGE, off the shared HWDGE device entirely.
  - a chain of wide dummy matmuls from t~0.5us keeps the PE p-state ramp
    warm so the real convs run at full clock (0.42 ns/row, not 0.83).
  - attention for all 8 patches runs as one batched 128x128 matmul pair; the
    block-diagonal -30000 mask is pre-accumulated into the scores PSUM via a
    single K=9 matmul; the V tail is split so only a 4-chunk conv, one matmul
    pair and one small DMA sit after the last weight byte lands.
"""
import os
import sys

sys.path.insert(0, "/opt/trn_rl_repo")

import numpy as np

EPS = 1e-5
D_IN, D, B, HW, P = 2048, 256, 4, 16, 4
NCHUNK = D_IN // 128  # 16
N_CORES = 8
MASK_NEG = 30000.0
F8MAX = 15.5  # TRN float8e3 (e3m4) max normal
X_DT = os.environ.get("KERNEL_XDT", "float8e3")  # bfloat16 | float8e3
N_WARM = int(os.environ.get("KERNEL_WARM", "13"))

_CACHE = {}

# rows aux layout (bf16): [1, 896]
_R_ONES = slice(0, 128)
_R_BD = slice(128, 384)            # d-conv bias (BN+scale folded)
_R_BETA = slice(384, 640)          # out-BN beta (scale folded)
_R_BQ = slice(640, 896)            # bq0|bq1 rows [1,128] each
ROWS_LEN = 896
# aux bf16 [128, 896]: posA_sv | posb_k | posb_j | identity
# combo f32 [128, 16]: R/(sx^2 sq sk) [0:8], 1/(sx sd) [8], 1/(sx sv) [9]


def _build_program(x_dt_name: str):
    """Build (and compile to BIR) the single-core SPMD Bass program."""
    import concourse.mybir as mybir
    import concourse.tile as tile
    from concourse import bacc

    xdt = getattr(mybir.dt, x_dt_name)
    wdt = mybir.dt.float8e3
    bdt = mybir.dt.bfloat16
    f32 = mybir.dt.float32

    nc = bacc.Bacc("TRN2", target_bir_lowering=False, debug=False,
                   num_devices=N_CORES)

    xcols = NCHUNK * 128
    x_d = nc.dram_tensor("x", [128, xcols], xdt, kind="ExternalInput")
    w_d = {name: nc.dram_tensor(name, [128, NCHUNK * 256], wdt,
                                kind="ExternalInput")
           for name in ("wd", "wv", "wk", "wq")}
    aux_d = nc.dram_tensor("aux", [128, 896], bdt, kind="ExternalInput")
    combo_d = nc.dram_tensor("combo", [128, 16], f32, kind="ExternalInput")
    rows_d = nc.dram_tensor("rows", [1, ROWS_LEN], bdt, kind="ExternalInput")
    mask9_d = nc.dram_tensor("mask9", [9, 256], bdt, kind="ExternalInput")
    out_d = nc.dram_tensor("xloc", [128, 256], f32, kind="ExternalOutput")

    with tile.TileContext(nc) as tc:
        with (
            tc.tile_pool(name="big", bufs=1) as big,
            tc.tile_pool(name="small", bufs=1) as small,
            tc.tile_pool(name="ps", bufs=1, space="PSUM") as ps,
            tc.tile_pool(name="ps2", bufs=2, space="PSUM") as ps2,
        ):
            xt = big.tile([128, xcols], xdt, tag="xt")
            wt = {n: big.tile([128, NCHUNK * 256], wdt, tag=n, name=n + "_t")
                  for n in ("wd", "wv", "wk", "wq")}
            aux = small.tile([128, 896], bdt, tag="aux")
            combo = small.tile([128, 16], f32, tag="combo")
            rows = small.tile([1, ROWS_LEN], bdt, tag="rows")
            mask9 = small.tile([9, 256], bdt, tag="mask9")

            # ---- PE p-state warmup: wide dummy matmuls back-to-back from
            # ~0.5us keep the tensor clock ramping while the stream lands ----
            if N_WARM:
                wsrc = small.tile([1, 256], bdt, tag="wsrc")
                nc.gpsimd.memset(wsrc[:], 0.0)
                warm_ps = ps2.tile([1, 256], f32, tag="post", name="warm_ps")
                for _ in range(N_WARM):
                    nc.tensor.matmul(warm_ps[:], wsrc[0:1, 0:1], wsrc[:],
                                     start=True, stop=True)

            # ---- DMA loads. Big tensors: SP HWDGE, compute order; x/wd
            # halves let the d conv start while the rest streams.
            # Small aux tensors: Pool SWDGE (no shared-HWDGE contention). ----
            # Pool SWDGE pipe: x half 1 (lands ~3.7us, well before the
            # d-conv tail needs it) + small latency-tolerant tensors.
            xh = xcols // 2
            nc.gpsimd.dma_start(xt[:, xh:xcols], x_d.ap()[:, xh:xcols])
            nc.gpsimd.dma_start(combo[:], combo_d.ap())
            nc.gpsimd.dma_start(mask9[:], mask9_d.ap())

            nc.sync.dma_start(wt["wd"][:, 0:2048], w_d["wd"].ap()[:, 0:2048])
            nc.sync.dma_start(xt[:, 0:xh], x_d.ap()[:, 0:xh])
            nc.sync.dma_start(wt["wd"][:, 2048:4096],
                              w_d["wd"].ap()[:, 2048:4096])
            nc.sync.dma_start(wt["wq"][:, 0:2048], w_d["wq"].ap()[:, 0:2048])
            nc.sync.dma_start(rows[:], rows_d.ap())
            nc.sync.dma_start(wt["wq"][:, 2048:4096],
                              w_d["wq"].ap()[:, 2048:4096])
            nc.sync.dma_start(aux[:], aux_d.ap())
            nc.sync.dma_start(wt["wk"][:, 0:2048], w_d["wk"].ap()[:, 0:2048])
            nc.sync.dma_start(wt["wk"][:, 2048:4096],
                              w_d["wk"].ap()[:, 2048:4096])
            nc.sync.dma_start(wt["wv"][:, 0:2048], w_d["wv"].ap()[:, 0:2048])
            nc.sync.dma_start(wt["wv"][:, 2048:3584],
                              w_d["wv"].ap()[:, 2048:3584])
            nc.sync.dma_start(wt["wv"][:, 3584:4096],
                              w_d["wv"].ap()[:, 3584:4096])

            posa = aux[:, 0:256]
            posbk = aux[:, 256:512]
            posbj = aux[:, 512:768]
            ident = aux[:, 768:896]
            R_ap = combo[:, 0:8]
            dscale = combo[:, 8:9]
            oscale = combo[:, 9:10]
            ones_r = rows[0:1, _R_ONES]

            # ---- conv PSUM accumulators ----
            d_ps = ps2.tile([128, 256], f32, tag="post", name="d_ps")
            kq_ps = [[ps.tile([128, 128], f32, tag=f"{n}{h}_ps", name=f"{n}{h}_ps")
                      for h in range(2)] for n in ("k", "q")]
            v_ps = [ps.tile([128, 128], f32, tag=f"v{g}_ps", name=f"v{g}_ps")
                    for g in range(2)]

            def a_conv_d(c0, c1):
                # A-layout [pix, outch], x chunk stationary
                for c in range(c0, c1):
                    nc.tensor.matmul(d_ps[:], xt[:, c * 128:(c + 1) * 128],
                                     wt["wd"][:, c * 256:(c + 1) * 256],
                                     start=(c == 0), stop=False)

            def b_conv(name, wi, h, brow, pre=None):
                # B-layout [outch, pix], weight chunk stationary; wk/wq packed
                # outch-half-major: col = h*2048 + c*128 + o. `pre` seeds the
                # PSUM group with a full [128,128] operand via the identity
                # rank-128 matmul; `brow` adds a rank-1 bias at the end.
                acc = kq_ps[wi][h]
                if pre is not None:
                    nc.tensor.matmul(acc[:], ident, pre, start=True, stop=False)
                for c in range(NCHUNK):
                    nc.tensor.matmul(
                        acc[:],
                        wt[name][:, h * 2048 + c * 128:h * 2048 + (c + 1) * 128],
                        xt[:, c * 128:(c + 1) * 128],
                        start=(pre is None and c == 0),
                        stop=(brow is None and c == NCHUNK - 1))
                if brow is not None:
                    nc.tensor.matmul(acc[:], brow, ones_r, start=False, stop=True)

            def v_conv(g, c0, c1):
                # A-layout half [pix, 128], wv packed outch-half-major
                for c in range(c0, c1):
                    nc.tensor.matmul(
                        v_ps[g][:], xt[:, c * 128:(c + 1) * 128],
                        wt["wv"][:, g * 2048 + c * 128:g * 2048 + (c + 1) * 128],
                        start=(c == 0), stop=(c == NCHUNK - 1))

            AF = mybir.ActivationFunctionType

            # ---- PE: d conv (chunks track the x/wd half DMAs), bias last ----
            a_conv_d(0, 8)
            a_conv_d(8, NCHUNK)
            nc.tensor.matmul(d_ps[:], ones_r, rows[0:1, _R_BD],
                             start=False, stop=True)

            # ---- d path: relu -> exp(scale*in, accum); normalization folds
            # into R (S = einc.T @ (R*dsuminv)), skipping the incx tile ----
            inc = small.tile([128, 256], f32, tag="inc")
            nc.scalar.activation(inc[:], d_ps[:], AF.Relu)
            einc = small.tile([128, 256], f32, tag="einc")
            dsum = small.tile([128, 1], f32, tag="dsum")
            nc.scalar.activation(einc[:], inc[:], AF.Exp, scale=dscale,
                                 accum_out=dsum[:])
            dsuminv = small.tile([128, 1], f32, tag="dsuminv")
            nc.vector.reciprocal(dsuminv[:], dsum[:])
            sR = small.tile([128, 8], f32, tag="sR")
            nc.vector.tensor_scalar_mul(sR[:], R_ap, dsuminv[:, 0:1])
            s_ps = ps2.tile([128, 16], f32, tag="post", name="s_ps")
            sT = small.tile([128, 16], f32, tag="sT")
            for h in range(2):
                nc.tensor.matmul(s_ps[:, h * 8:(h + 1) * 8],
                                 einc[:, h * 128:(h + 1) * 128],
                                 sR[:], start=True, stop=True,
                                 skip_group_check=(h == 1))
            nc.vector.tensor_copy(sT[:], s_ps[:])

            # ---- scores PSUM: block mask first (one K=9 matmul) ----
            sc_ps = ps2.tile([128, 128], f32, tag="post", name="sc_ps")
            nc.tensor.matmul(sc_ps[:], mask9[:, 0:128], mask9[:, 128:256],
                             start=True, stop=False)

            # ---- q convs (B-layout); jtmp = q_psum * S broadcast ----
            jtmp = [small.tile([128, 128], bdt, tag=f"jtmp{h}", name=f"jtmp{h}")
                    for h in range(2)]
            for h in range(2):
                b_conv("wq", 1, h, rows[0:1, 640 + h * 128:768 + h * 128])
                s_bcast = sT[:, h * 8:(h + 1) * 8].unsqueeze(2).broadcast_to((128, 8, 16))
                q3d = kq_ps[1][h][:].rearrange("p (b w) -> p b w", b=8)
                j3d = jtmp[h][:].rearrange("p (b w) -> p b w", b=8)
                nc.vector.tensor_tensor(j3d, q3d, s_bcast, op=mybir.AluOpType.mult)

            # ---- k convs: posbk (incl bk) seeds the PSUM group, so Kp is a
            # plain PSUM->SBUF copy on the scalar engine ----
            kp = [small.tile([128, 128], bdt, tag=f"kp{h}", name=f"kp{h}")
                  for h in range(2)]
            for h in range(2):
                b_conv("wk", 0, h, None, pre=posbk[:, h * 128:(h + 1) * 128])
                nc.scalar.activation(kp[h][:], kq_ps[0][h][:], AF.Copy)

            # ---- scores + v convs interleaved on PE. The posbj term rides
            # the scores PSUM as Kp.T @ posbj, so no vector add for J. ----
            nc.tensor.matmul(sc_ps[:], kp[0][:], posbj[:, 0:128],
                             start=False, stop=False)
            nc.tensor.matmul(sc_ps[:], kp[0][:], jtmp[0][:],
                             start=False, stop=False)
            v_conv(0, 0, 10)
            nc.tensor.matmul(sc_ps[:], kp[1][:], posbj[:, 128:256],
                             start=False, stop=False)
            nc.tensor.matmul(sc_ps[:], kp[1][:], jtmp[1][:],
                             start=False, stop=True)
            v_conv(0, 10, NCHUNK)
            v_conv(1, 0, 12)
            v_conv(1, 12, NCHUNK)

            nmx = small.tile([128, 1], f32, tag="nmx")
            nc.vector.reduce_max(nmx[:], sc_ps[:], axis=mybir.AxisListType.X,
                                 negate=True)
            e_t = small.tile([128, 128], f32, tag="e_t")
            den = small.tile([128, 1], f32, tag="den")
            nc.scalar.activation(e_t[:], sc_ps[:], AF.Exp, bias=nmx[:, 0:1],
                                 accum_out=den[:])
            vpt = small.tile([128, 256], bdt, tag="vpt")
            xloc = small.tile([128, 256], f32, tag="xloc")
            nc.vector.tensor_tensor(vpt[:, 0:128], v_ps[0][:], posa[:, 0:128],
                                    op=mybir.AluOpType.add)
            deninv = small.tile([128, 1], f32, tag="deninv")
            nc.vector.reciprocal(deninv[:], den[:])
            att = small.tile([128, 128], bdt, tag="att")
            nc.vector.tensor_scalar_mul(att[:], e_t[:], deninv[:, 0:1])
            nc.vector.tensor_tensor(vpt[:, 128:256], v_ps[1][:],
                                    posa[:, 128:256], op=mybir.AluOpType.add)
            att_ps = [ps.tile([128, 128], f32, tag=f"q{g}_ps",
                              name=f"att_ps{g}") for g in range(2)]
            for g in range(2):
                gs = slice(g * 128, (g + 1) * 128)
                nc.tensor.matmul(att_ps[g][:], ones_r,
                                 rows[0:1, 384 + g * 128:384 + (g + 1) * 128],
                                 start=True, stop=False)
                nc.tensor.matmul(att_ps[g][:], att[:], vpt[:, gs], start=False,
                                 stop=True)
            nc.vector.tensor_scalar_mul(xloc[:, 0:128], att_ps[0][:], oscale)
            nc.scalar.activation(xloc[:, 128:256], att_ps[1][:], AF.Copy,
                                 scale=oscale)
            nc.scalar.dma_start(out_d.ap()[:], xloc[:])

    nc.compile()
    return nc


def _fold_bn(w, b, g, beta, m, v):
    s = g / np.sqrt(v + EPS)
    return (w * s[:, None]).astype(np.float32), (s * (b - m) + beta).astype(np.float32)


def _np_dt(name):
    import ml_dtypes
    if name == "bfloat16":
        return ml_dtypes.bfloat16
    if name == "float8e3":
        return ml_dtypes.float8_e3m4
    return np.float32


def _prep(inputs):
    """Host-side prep: BN folds, fp8 quantization + per-core input maps."""
    import ml_dtypes
    bf = ml_dtypes.bfloat16
    f8 = ml_dtypes.float8_e3m4
    x_np_dt = _np_dt(X_DT)

    inp = {k: np.asarray(v, dtype=np.float32) for k, v in inputs.items()}
    x, pos = inp["x"], inp["pos"]
    wk, bk = _fold_bn(inp["wk"], inp["bk"], inp["gk"], inp["betak"], inp["mk"], inp["vk"])
    wq, bq = _fold_bn(inp["wq"], inp["bq"], inp["gq"], inp["betaq"], inp["mq"], inp["vq"])
    wv, bv = _fold_bn(inp["wv"], inp["bv"], inp["gv"], inp["betav"], inp["mv"], inp["vv"])
    wd, bd = _fold_bn(inp["wd"], inp["bd"], inp["gd"], inp["betad"], inp["md"], inp["vd"])
    so = (inp["go"] / np.sqrt(inp["vo"] + EPS)).astype(np.float32)
    beta_o = (inp["beto"] - inp["mo"] * so).astype(np.float32)
    wv = wv * so[:, None]
    bv = bv * so

    # per-tensor fp8 scales; s_x = 1 when x stays bf16
    s_x = float(F8MAX / np.abs(x).max()) if X_DT == "float8e3" else 1.0
    sc = {n: float(F8MAX / np.abs(w).max())
          for n, w in (("d", wd), ("k", wk), ("q", wq), ("v", wv))}

    def quant(w, s):
        return np.clip(w * s, -F8MAX, F8MAX).astype(f8)

    def wpack_cmaj(w):  # chunk-major: [256 out, 2048 in] -> [128, (c,256)]
        wt = w.T.reshape(NCHUNK, 128, 256).transpose(1, 0, 2).reshape(128, -1)
        return np.ascontiguousarray(wt)

    def wpack_hmaj(w):  # outch-half-major: [256 out, 2048 in] -> [128,(h,c,128)]
        wt = w.T.reshape(NCHUNK, 128, 2, 128).transpose(1, 2, 0, 3).reshape(128, -1)
        return np.ascontiguousarray(wt)

    w_packed = {"wd": wpack_cmaj(quant(wd, sc["d"])),
                "wv": wpack_hmaj(quant(wv, sc["v"])),
                "wk": wpack_hmaj(quant(wk, sc["k"])),
                "wq": wpack_hmaj(quant(wq, sc["q"]))}

    p_idx = np.arange(128)
    R = np.zeros((128, 8), np.float32)
    R[p_idx, (p_idx // 64) * 4 + (p_idx % 16) // 4] = 1.0
    R /= (s_x * s_x * sc["q"] * sc["k"])
    pix_patch = (p_idx // 64) * 4 + (p_idx % 64) // 16
    blk_ind = (pix_patch[None, :] == np.arange(8)[:, None]).astype(np.float32)

    rows = np.zeros((1, ROWS_LEN), np.float32)
    rows[0, _R_ONES] = 1.0
    rows[0, _R_BD] = bd * (s_x * sc["d"])
    rows[0, _R_BETA] = beta_o * (s_x * sc["v"])
    rows[0, _R_BQ] = bq * (s_x * sc["q"])
    rows = rows.astype(bf)

    mask9 = np.zeros((9, 256), np.float32)
    mask9[0, 0:128] = 1.0
    mask9[0, 128:256] = -MASK_NEG
    mask9[1:9, 0:128] = blk_ind
    mask9[1:9, 128:256] = blk_ind * MASK_NEG
    mask9 = mask9.astype(bf)

    units = [(b, i) for b in range(B) for i in range(P)]
    in_maps = []
    for core in range(N_CORES):
        cu = units[2 * core:2 * core + 2]
        x_sb = np.empty((128, NCHUNK, 128), np.float32)
        pos_A = np.empty((128, 256), np.float32)
        posb_sb = np.empty((128, 256), np.float32)
        for u, (b, i) in enumerate(cu):
            # [c, ph, jp, pw] -> patch-major pixel (jp, ph, pw)
            xs = x[b, :, 4 * i:4 * i + 4, :].reshape(D_IN, 4, 4, 4)
            xs = xs.transpose(0, 2, 1, 3).reshape(D_IN, 64)
            x_sb[:, :, 64 * u:64 * u + 64] = xs.reshape(NCHUNK, 128, 64).transpose(1, 0, 2)
            ps_ = pos[b, :, 4 * i:4 * i + 4, :].reshape(D, 4, 4, 4).transpose(0, 2, 1, 3).reshape(D, 64)
            pos_A[64 * u:64 * u + 64, :] = ps_.T
            posb_sb[:, 64 * u:64 * u + 64] = ps_[0:128]
            posb_sb[:, 128 + 64 * u:128 + 64 * u + 64] = ps_[128:256]
        pos_A_sov = (pos_A * so[None, :] + bv[None, :]) * (s_x * sc["v"])
        # fold bk (per out-channel) into posbk: posb layout [ch, h*128+pix]
        posb_k = posb_sb.copy()
        for h in range(2):
            posb_k[:, h * 128:(h + 1) * 128] += bk[h * 128:(h + 1) * 128][:, None]
        posb_k = posb_k * (s_x * sc["k"])
        posb_j = posb_sb / (s_x * sc["k"])
        aux = np.concatenate([pos_A_sov, posb_k, posb_j,
                              np.eye(128, dtype=np.float32)], axis=1).astype(bf)
        combo = np.zeros((128, 16), np.float32)
        combo[:, 0:8] = R
        combo[:, 8] = 1.0 / (s_x * sc["d"])
        combo[:, 9] = 1.0 / (s_x * sc["v"])
        if X_DT == "float8e3":
            x_core = np.clip(x_sb.reshape(128, -1) * s_x,
                             -F8MAX, F8MAX).astype(f8)
        else:
            x_core = x_sb.reshape(128, -1).astype(bf)
        in_maps.append({
            "x": np.ascontiguousarray(x_core),
            "wd": w_packed["wd"], "wv": w_packed["wv"],
            "wk": w_packed["wk"], "wq": w_packed["wq"],
            "aux": np.ascontiguousarray(aux),
            "combo": combo, "rows": rows, "mask9": mask9,
        })
    return in_maps, units


def _run_device(nc, in_maps):
    from concourse.bass_utils import run_bass_kernel_spmd
    return run_bass_kernel_spmd(nc, in_maps, list(range(N_CORES))).results


def _subproc_main(inp_path, out_path):
    import pickle
    with open(inp_path, "rb") as f:
        in_maps = pickle.load(f)
    nc = _build_program(X_DT)
    res = _run_device(nc, in_maps)
    with open(out_path, "wb") as f:
        pickle.dump(res, f)


def _run_via_subprocess(in_maps):
    import pickle
    import subprocess
    import tempfile
    here = os.path.dirname(os.path.abspath(__file__))
    last = None
    for _ in range(2):
        with tempfile.TemporaryDirectory() as td:
            inp = os.path.join(td, "in.pkl")
            outp = os.path.join(td, "out.pkl")
            with open(inp, "wb") as f:
                pickle.dump(in_maps, f)
            code = (f"import sys; sys.path.insert(0, {here!r}); "
                    f"import kernel; kernel._subproc_main({inp!r}, {outp!r})")
            try:
                r = subprocess.run([sys.executable, "-c", code], timeout=1800)
                if r.returncode == 0 and os.path.exists(outp):
                    with open(outp, "rb") as f:
                        return pickle.load(f)
                last = RuntimeError(f"subprocess rc={r.returncode}")
            except Exception as e:  # noqa: BLE001
                last = e
    raise RuntimeError(f"device execution failed after retries: {last}")


def kernel(**inputs) -> np.ndarray:
    key = ("prog", X_DT)
    if key not in _CACHE:
        _CACHE[key] = _build_program(X_DT)
    nc = _CACHE[key]

    in_maps, units = _prep(inputs)
    try:
        results = _run_device(nc, in_maps)
    except Exception:
        # A crashed NEFF execution can poison this process's jax runtime
        # (NRT_EXEC_UNIT_UNRECOVERABLE); a fresh process recovers reliably.
        results = _run_via_subprocess(in_maps)

    x_loc = np.zeros((B, D, HW, HW), np.float32)
    for core in range(N_CORES):
        xl = results[core]["xloc"]  # [128 pix, 256 c]
        for u, (b, i) in enumerate(units[2 * core:2 * core + 2]):
            blk = xl[64 * u:64 * u + 64, :].reshape(4, 4, 4, D).transpose(3, 1, 0, 2)
            x_loc[b, :, 4 * i:4 * i + 4, :] = blk.reshape(D, 4, 16)
    return np.concatenate([np.asarray(inputs["x"], np.float32), x_loc], axis=1)


# revision 21
# speedup vs baseline: 1.0114x; 1.0114x over previous
"""Trainium2 Bass kernel for nn_Block_Attention_3 (sparse_attention).

Contract: kernel(**inputs) takes FULL fp32 inputs (as in reference.setup_inputs())
and returns the FULL (4, 2304, 16, 16) fp32 output.

Strategy (zero-collective position sharding + fp8 weight streaming):
  The image is 16x16 = 4x4 grid of 4x4 patches. All cross-position coupling in
  the block stays within one (batch, patch-row) group, so the 16 units (b, i)
  shard cleanly across 8 cores, 2 units/core, with weights replicated.
  Replication is the whole cost: the kernel is DMA-stream-bound on the four
  2048x256 conv weight matrices. They are streamed as float8e3 (e3m4) with
  per-tensor scales, halving weight traffic vs bf16; x streams as e3m4 too.
  Every descale folds into host-prepared operands (R, pos copies, bias rows)
  or rides per-partition scale columns, so the compiled program is
  input-independent and exact up to the fp8 rounding itself.

Per-core pipeline (single Bass program, SPMD over 8 cores):
  - inference BN folded into conv weights/biases on host; out-BN scale into the
    V path; v-bias + k-bias ride the pos operands; bd/bq/beta enter as rank-1
    matmuls at the END of each PSUM accumulation group.
  - pixels are laid out patch-major: pix = u*64 + 16*jp + 4*ph + pw.
  - d,v convs accumulate in A-layout [pix, outch]; k,q convs in B-layout
    [outch, pix] so the scores matmul needs no transposes.
  - big tensors stream via SP-queue HWDGE (9 DMAs, sized so desc-gen stays
    ahead of the 360 B/ns transfer stream); small aux tensors go via Pool
    SWDGE, off the shared HWDGE device entirely.
  - a chain of wide dummy matmuls from t~0.5us keeps the PE p-state ramp
    warm so the real convs run at full clock (0.42 ns/row, not 0.83).
  - attention for all 8 patches runs as one batched 128x128 matmul pair; the
    block-diagonal -30000 mask is pre-accumulated into the scores PSUM via a
    single K=9 matmul; the V tail is split so only a 4-chunk conv, one matmul
    pair and one small DMA sit after the last weight byte lands.
"""
import os
import sys

sys.path.insert(0, "/opt/trn_rl_repo")

import numpy as np

EPS = 1e-5
D_IN, D, B, HW, P = 2048, 256, 4, 16, 4
NCHUNK = D_IN // 128  # 16
N_CORES = 8
MASK_NEG = 30000.0
F8MAX = 15.5  # TRN float8e3 (e3m4) max normal
X_DT = os.environ.get("KERNEL_XDT", "float8e3")  # bfloat16 | float8e3
N_WARM = int(os.environ.get("KERNEL_WARM", "13"))

_CACHE = {}

# rows aux layout (bf16): [1, 896]
_R_ONES = slice(0, 128)
_R_BD = slice(128, 384)            # d-conv bias (BN+scale folded)
_R_BETA = slice(384, 640)          # out-BN beta (scale folded)
_R_BQ = slice(640, 896)            # bq0|bq1 rows [1,128] each
ROWS_LEN = 896
# aux bf16 [128, 896]: posA_sv | posb_k | posb_j | identity
# combo f32 [128, 16]: R/(sx^2 sq sk) [0:8], 1/(sx sd) [8], 1/(sx sv) [9]


def _build_program(x_dt_name: str):
    """Build (and compile to BIR) the single-core SPMD Bass program."""
    import concourse.mybir as mybir
    import concourse.tile as tile
    from concourse import bacc

    xdt = getattr(mybir.dt, x_dt_name)
    wdt = mybir.dt.float8e3
    bdt = mybir.dt.bfloat16
    f32 = mybir.dt.float32

    nc = bacc.Bacc("TRN2", target_bir_lowering=False, debug=False,
                   num_devices=N_CORES)

    xcols = NCHUNK * 128
    x_d = nc.dram_tensor("x", [128, xcols], xdt, kind="ExternalInput")
    w_d = {name: nc.dram_tensor(name, [128, NCHUNK * 256], wdt,
                                kind="ExternalInput")
           for name in ("wd", "wv", "wk", "wq")}
    aux_d = nc.dram_tensor("aux", [128, 896], bdt, kind="ExternalInput")
    combo_d = nc.dram_tensor("combo", [128, 16], f32, kind="ExternalInput")
    rows_d = nc.dram_tensor("rows", [1, ROWS_LEN], bdt, kind="ExternalInput")
    mask9_d = nc.dram_tensor("mask9", [9, 256], bdt, kind="ExternalInput")
    out_d = nc.dram_tensor("xloc", [128, 256], f32, kind="ExternalOutput")

    with tile.TileContext(nc) as tc:
        with (
            tc.tile_pool(name="big", bufs=1) as big,
            tc.tile_pool(name="small", bufs=1) as small,
            tc.tile_pool(name="ps", bufs=1, space="PSUM") as ps,
            tc.tile_pool(name="ps2", bufs=2, space="PSUM") as ps2,
        ):
            xt = big.tile([128, xcols], xdt, tag="xt")
            wt = {n: big.tile([128, NCHUNK * 256], wdt, tag=n, name=n + "_t")
                  for n in ("wd", "wv", "wk", "wq")}
            aux = small.tile([128, 896], bdt, tag="aux")
            combo = small.tile([128, 16], f32, tag="combo")
            rows = small.tile([1, ROWS_LEN], bdt, tag="rows")
            mask9 = small.tile([9, 256], bdt, tag="mask9")

            # ---- PE p-state warmup: wide dummy matmuls back-to-back from
            # ~0.5us keep the tensor clock ramping while the stream lands ----
            if N_WARM:
                wsrc = small.tile([1, 256], bdt, tag="wsrc")
                nc.gpsimd.memset(wsrc[:], 0.0)
                warm_ps = ps2.tile([1, 256], f32, tag="post", name="warm_ps")
                for _ in range(N_WARM):
                    nc.tensor.matmul(warm_ps[:], wsrc[0:1, 0:1], wsrc[:],
                                     start=True, stop=True)

            # ---- DMA loads. Big tensors: SP HWDGE, compute order; x/wd
            # halves let the d conv start while the rest streams.
            # Small aux tensors: Pool SWDGE (no shared-HWDGE contention). ----
            # Pool SWDGE pipe: x half 1 (lands ~3.7us, well before the
            # d-conv tail needs it) + small latency-tolerant tensors.
            xh = xcols // 2
            nc.gpsimd.dma_start(xt[:, xh:xcols], x_d.ap()[:, xh:xcols])
            nc.gpsimd.dma_start(combo[:], combo_d.ap())
            nc.gpsimd.dma_start(mask9[:], mask9_d.ap())

            nc.sync.dma_start(wt["wd"][:, 0:2048], w_d["wd"].ap()[:, 0:2048])
            nc.sync.dma_start(xt[:, 0:xh], x_d.ap()[:, 0:xh])
            nc.sync.dma_start(wt["wd"][:, 2048:4096],
                              w_d["wd"].ap()[:, 2048:4096])
            nc.sync.dma_start(wt["wq"][:, 0:2048], w_d["wq"].ap()[:, 0:2048])
            nc.sync.dma_start(rows[:], rows_d.ap())
            nc.sync.dma_start(wt["wq"][:, 2048:4096],
                              w_d["wq"].ap()[:, 2048:4096])
            nc.sync.dma_start(aux[:], aux_d.ap())
            nc.sync.dma_start(wt["wk"][:, 0:2048], w_d["wk"].ap()[:, 0:2048])
            nc.sync.dma_start(wt["wk"][:, 2048:4096],
                              w_d["wk"].ap()[:, 2048:4096])
            nc.sync.dma_start(wt["wv"][:, 0:2048], w_d["wv"].ap()[:, 0:2048])
            nc.sync.dma_start(wt["wv"][:, 2048:3584],
                              w_d["wv"].ap()[:, 2048:3584])
            nc.sync.dma_start(wt["wv"][:, 3584:4096],
                              w_d["wv"].ap()[:, 3584:4096])

            posa = aux[:, 0:256]
            posbk = aux[:, 256:512]
            posbj = aux[:, 512:768]
            ident = aux[:, 768:896]
            R_ap = combo[:, 0:8]
            dscale = combo[:, 8:9]
            oscale = combo[:, 9:10]
            ones_r = rows[0:1, _R_ONES]

            # ---- conv PSUM accumulators ----
            d_ps = ps2.tile([128, 256], f32, tag="post", name="d_ps")
            kq_ps = [[ps.tile([128, 128], f32, tag=f"{n}{h}_ps", name=f"{n}{h}_ps")
                      for h in range(2)] for n in ("k", "q")]
            v_ps = [ps.tile([128, 128], f32, tag=f"v{g}_ps", name=f"v{g}_ps")
                    for g in range(2)]

            def a_conv_d(c0, c1):
                # A-layout [pix, outch], x chunk stationary
                for c in range(c0, c1):
                    nc.tensor.matmul(d_ps[:], xt[:, c * 128:(c + 1) * 128],
                                     wt["wd"][:, c * 256:(c + 1) * 256],
                                     start=(c == 0), stop=False)

            def b_conv(name, wi, h, brow, pre=None):
                # B-layout [outch, pix], weight chunk stationary; wk/wq packed
                # outch-half-major: col = h*2048 + c*128 + o. `pre` seeds the
                # PSUM group with a full [128,128] operand via the identity
                # rank-128 matmul; `brow` adds a rank-1 bias at the end.
                acc = kq_ps[wi][h]
                if pre is not None:
                    nc.tensor.matmul(acc[:], ident, pre, start=True, stop=False)
                for c in range(NCHUNK):
                    nc.tensor.matmul(
                        acc[:],
                        wt[name][:, h * 2048 + c * 128:h * 2048 + (c + 1) * 128],
                        xt[:, c * 128:(c + 1) * 128],
                        start=(pre is None and c == 0),
                        stop=(brow is None and c == NCHUNK - 1))
                if brow is not None:
                    nc.tensor.matmul(acc[:], brow, ones_r, start=False, stop=True)

            def v_conv(g, c0, c1):
                # A-layout half [pix, 128], wv packed outch-half-major
                for c in range(c0, c1):
                    nc.tensor.matmul(
                        v_ps[g][:], xt[:, c * 128:(c + 1) * 128],
                        wt["wv"][:, g * 2048 + c * 128:g * 2048 + (c + 1) * 128],
                        start=(c == 0), stop=(c == NCHUNK - 1))

            AF = mybir.ActivationFunctionType

            # ---- PE: d conv (chunks track the x/wd half DMAs), bias last ----
            a_conv_d(0, 8)
            a_conv_d(8, NCHUNK)
            nc.tensor.matmul(d_ps[:], ones_r, rows[0:1, _R_BD],
                             start=False, stop=True)

            # ---- d path: relu -> exp(scale*in, accum); normalization folds
            # into R (S = einc.T @ (R*dsuminv)), skipping the incx tile ----
            inc = small.tile([128, 256], f32, tag="inc")
            nc.scalar.activation(inc[:], d_ps[:], AF.Relu)
            einc = small.tile([128, 256], f32, tag="einc")
            dsum = small.tile([128, 1], f32, tag="dsum")
            nc.scalar.activation(einc[:], inc[:], AF.Exp, scale=dscale,
                                 accum_out=dsum[:])
            dsuminv = small.tile([128, 1], f32, tag="dsuminv")
            nc.vector.reciprocal(dsuminv[:], dsum[:])
            sR = small.tile([128, 8], f32, tag="sR")
            nc.vector.tensor_scalar_mul(sR[:], R_ap, dsuminv[:, 0:1])
            s_ps = ps2.tile([128, 16], f32, tag="post", name="s_ps")
            sT = small.tile([128, 16], f32, tag="sT")
            for h in range(2):
                nc.tensor.matmul(s_ps[:, h * 8:(h + 1) * 8],
                                 einc[:, h * 128:(h + 1) * 128],
                                 sR[:], start=True, stop=True,
                                 skip_group_check=(h == 1))
            nc.vector.tensor_copy(sT[:], s_ps[:])

            # ---- scores PSUM: block mask first (one K=9 matmul) ----
            sc_ps = ps2.tile([128, 128], f32, tag="post", name="sc_ps")
            nc.tensor.matmul(sc_ps[:], mask9[:, 0:128], mask9[:, 128:256],
                             start=True, stop=False)

            # ---- q convs (B-layout); jtmp = q_psum * S broadcast ----
            jtmp = [small.tile([128, 128], bdt, tag=f"jtmp{h}", name=f"jtmp{h}")
                    for h in range(2)]
            for h in range(2):
                b_conv("wq", 1, h, rows[0:1, 640 + h * 128:768 + h * 128])
                s_bcast = sT[:, h * 8:(h + 1) * 8].unsqueeze(2).broadcast_to((128, 8, 16))
                q3d = kq_ps[1][h][:].rearrange("p (b w) -> p b w", b=8)
                j3d = jtmp[h][:].rearrange("p (b w) -> p b w", b=8)
                nc.vector.tensor_tensor(j3d, q3d, s_bcast, op=mybir.AluOpType.mult)

            # ---- k convs: posbk (incl bk) seeds the PSUM group, so Kp is a
            # plain PSUM->SBUF copy on the scalar engine ----
            kp = [small.tile([128, 128], bdt, tag=f"kp{h}", name=f"kp{h}")
                  for h in range(2)]
            for h in range(2):
                b_conv("wk", 0, h, None, pre=posbk[:, h * 128:(h + 1) * 128])
                nc.scalar.activation(kp[h][:], kq_ps[0][h][:], AF.Copy)

            # ---- scores + v convs interleaved on PE. The posbj term rides
            # the scores PSUM as Kp.T @ posbj, so no vector add for J. ----
            nc.tensor.matmul(sc_ps[:], kp[0][:], posbj[:, 0:128],
                             start=False, stop=False)
            nc.tensor.matmul(sc_ps[:], kp[0][:], jtmp[0][:],
                             start=False, stop=False)
            v_conv(0, 0, 10)
            nc.tensor.matmul(sc_ps[:], kp[1][:], posbj[:, 128:256],
                             start=False, stop=False)
            nc.tensor.matmul(sc_ps[:], kp[1][:], jtmp[1][:],
                             start=False, stop=True)
            v_conv(0, 10, NCHUNK)
            v_conv(1, 0, 12)
            v_conv(1, 12, NCHUNK)

            nmx = small.tile([128, 1], f32, tag="nmx")
            nc.vector.reduce_max(nmx[:], sc_ps[:], axis=mybir.AxisListType.X,
                                 negate=True)
            e_t = small.tile([128, 128], f32, tag="e_t")
            den = small.tile([128, 1], f32, tag="den")
            nc.scalar.activation(e_t[:], sc_ps[:], AF.Exp, bias=nmx[:, 0:1],
                                 accum_out=den[:])
            vpt = small.tile([128, 256], bdt, tag="vpt")
            xloc = small.tile([128, 256], f32, tag="xloc")
            nc.vector.tensor_tensor(vpt[:, 0:128], v_ps[0][:], posa[:, 0:128],
                                    op=mybir.AluOpType.add)
            deninv = small.tile([128, 1], f32, tag="deninv")
            nc.vector.reciprocal(deninv[:], den[:])
            att = small.tile([128, 128], bdt, tag="att")
            nc.vector.tensor_scalar_mul(att[:], e_t[:], deninv[:, 0:1])
            nc.vector.tensor_tensor(vpt[:, 128:256], v_ps[1][:],
                                    posa[:, 128:256], op=mybir.AluOpType.add)
            att_ps = [ps.tile([128, 128], f32, tag=f"q{g}_ps",
                              name=f"att_ps{g}") for g in range(2)]
            for g in range(2):
                gs = slice(g * 128, (g + 1) * 128)
                nc.tensor.matmul(att_ps[g][:], ones_r,
                                 rows[0:1, 384 + g * 128:384 + (g + 1) * 128],
                                 start=True, stop=False)
                nc.tensor.matmul(att_ps[g][:], att[:], vpt[:, gs], start=False,
                                 stop=True)
            nc.scalar.activation(xloc[:, 0:128], att_ps[0][:], AF.Copy,
                                 scale=oscale)
            nc.vector.tensor_scalar_mul(xloc[:, 128:256], att_ps[1][:], oscale)
            nc.sync.dma_start(out_d.ap()[:], xloc[:])

    nc.compile()
    return nc


def _fold_bn(w, b, g, beta, m, v):
    s = g / np.sqrt(v + EPS)
    return (w * s[:, None]).astype(np.float32), (s * (b - m) + beta).astype(np.float32)


def _np_dt(name):
    import ml_dtypes
    if name == "bfloat16":
        return ml_dtypes.bfloat16
    if name == "float8e3":
        return ml_dtypes.float8_e3m4
    return np.float32


def _prep(inputs):
    """Host-side prep: BN folds, fp8 quantization + per-core input maps."""
    import ml_dtypes
    bf = ml_dtypes.bfloat16
    f8 = ml_dtypes.float8_e3m4
    x_np_dt = _np_dt(X_DT)

    inp = {k: np.asarray(v, dtype=np.float32) for k, v in inputs.items()}
    x, pos = inp["x"], inp["pos"]
    wk, bk = _fold_bn(inp["wk"], inp["bk"], inp["gk"], inp["betak"], inp["mk"], inp["vk"])
    wq, bq = _fold_bn(inp["wq"], inp["bq"], inp["gq"], inp["betaq"], inp["mq"], inp["vq"])
    wv, bv = _fold_bn(inp["wv"], inp["bv"], inp["gv"], inp["betav"], inp["mv"], inp["vv"])
    wd, bd = _fold_bn(inp["wd"], inp["bd"], inp["gd"], inp["betad"], inp["md"], inp["vd"])
    so = (inp["go"] / np.sqrt(inp["vo"] + EPS)).astype(np.float32)
    beta_o = (inp["beto"] - inp["mo"] * so).astype(np.float32)
    wv = wv * so[:, None]
    bv = bv * so

    # per-tensor fp8 scales; s_x = 1 when x stays bf16
    s_x = float(F8MAX / np.abs(x).max()) if X_DT == "float8e3" else 1.0
    sc = {n: float(F8MAX / np.abs(w).max())
          for n, w in (("d", wd), ("k", wk), ("q", wq), ("v", wv))}

    def quant(w, s):
        return np.clip(w * s, -F8MAX, F8MAX).astype(f8)

    def wpack_cmaj(w):  # chunk-major: [256 out, 2048 in] -> [128, (c,256)]
        wt = w.T.reshape(NCHUNK, 128, 256).transpose(1, 0, 2).reshape(128, -1)
        return np.ascontiguousarray(wt)

    def wpack_hmaj(w):  # outch-half-major: [256 out, 2048 in] -> [128,(h,c,128)]
        wt = w.T.reshape(NCHUNK, 128, 2, 128).transpose(1, 2, 0, 3).reshape(128, -1)
        return np.ascontiguousarray(wt)

    w_packed = {"wd": wpack_cmaj(quant(wd, sc["d"])),
                "wv": wpack_hmaj(quant(wv, sc["v"])),
                "wk": wpack_hmaj(quant(wk, sc["k"])),
                "wq": wpack_hmaj(quant(wq, sc["q"]))}

    p_idx = np.arange(128)
    R = np.zeros((128, 8), np.float32)
    R[p_idx, (p_idx // 64) * 4 + (p_idx % 16) // 4] = 1.0
    R /= (s_x * s_x * sc["q"] * sc["k"])
    pix_patch = (p_idx // 64) * 4 + (p_idx % 64) // 16
    blk_ind = (pix_patch[None, :] == np.arange(8)[:, None]).astype(np.float32)

    rows = np.zeros((1, ROWS_LEN), np.float32)
    rows[0, _R_ONES] = 1.0
    rows[0, _R_BD] = bd * (s_x * sc["d"])
    rows[0, _R_BETA] = beta_o * (s_x * sc["v"])
    rows[0, _R_BQ] = bq * (s_x * sc["q"])
    rows = rows.astype(bf)

    mask9 = np.zeros((9, 256), np.float32)
    mask9[0, 0:128] = 1.0
    mask9[0, 128:256] = -MASK_NEG
    mask9[1:9, 0:128] = blk_ind
    mask9[1:9, 128:256] = blk_ind * MASK_NEG
    mask9 = mask9.astype(bf)

    units = [(b, i) for b in range(B) for i in range(P)]
    in_maps = []
    for core in range(N_CORES):
        cu = units[2 * core:2 * core + 2]
        x_sb = np.empty((128, NCHUNK, 128), np.float32)
        pos_A = np.empty((128, 256), np.float32)
        posb_sb = np.empty((128, 256), np.float32)
        for u, (b, i) in enumerate(cu):
            # [c, ph, jp, pw] -> patch-major pixel (jp, ph, pw)
            xs = x[b, :, 4 * i:4 * i + 4, :].reshape(D_IN, 4, 4, 4)
            xs = xs.transpose(0, 2, 1, 3).reshape(D_IN, 64)
            x_sb[:, :, 64 * u:64 * u + 64] = xs.reshape(NCHUNK, 128, 64).transpose(1, 0, 2)
            ps_ = pos[b, :, 4 * i:4 * i + 4, :].reshape(D, 4, 4, 4).transpose(0, 2, 1, 3).reshape(D, 64)
            pos_A[64 * u:64 * u + 64, :] = ps_.T
            posb_sb[:, 64 * u:64 * u + 64] = ps_[0:128]
            posb_sb[:, 128 + 64 * u:128 + 64 * u + 64] = ps_[128:256]
        pos_A_sov = (pos_A * so[None, :] + bv[None, :]) * (s_x * sc["v"])
        # fold bk (per out-channel) into posbk: posb layout [ch, h*128+pix]
        posb_k = posb_sb.copy()
        for h in range(2):
            posb_k[:, h * 128:(h + 1) * 128] += bk[h * 128:(h + 1) * 128][:, None]
        posb_k = posb_k * (s_x * sc["k"])
        posb_j = posb_sb / (s_x * sc["k"])
        aux = np.concatenate([pos_A_sov, posb_k, posb_j,
                              np.eye(128, dtype=np.float32)], axis=1).astype(bf)
        combo = np.zeros((128, 16), np.float32)
        combo[:, 0:8] = R
        combo[:, 8] = 1.0 / (s_x * sc["d"])
        combo[:, 9] = 1.0 / (s_x * sc["v"])
        if X_DT == "float8e3":
            x_core = np.clip(x_sb.reshape(128, -1) * s_x,
                             -F8MAX, F8MAX).astype(f8)
        else:
            x_core = x_sb.reshape(128, -1).astype(bf)
        in_maps.append({
            "x": np.ascontiguousarray(x_core),
            "wd": w_packed["wd"], "wv": w_packed["wv"],
            "wk": w_packed["wk"], "wq": w_packed["wq"],
            "aux": np.ascontiguousarray(aux),
            "combo": combo, "rows": rows, "mask9": mask9,
        })
    return in_maps, units


def _run_device(nc, in_maps):
    from concourse.bass_utils import run_bass_kernel_spmd
    return run_bass_kernel_spmd(nc, in_maps, list(range(N_CORES))).results


def _subproc_main(inp_path, out_path):
    import pickle
    with open(inp_path, "rb") as f:
        in_maps = pickle.load(f)
    nc = _build_program(X_DT)
    res = _run_device(nc, in_maps)
    with open(out_path, "wb") as f:
        pickle.dump(res, f)


def _run_via_subprocess(in_maps):
    import pickle
    import subprocess
    import tempfile
    here = os.path.dirname(os.path.abspath(__file__))
    last = None
    for _ in range(2):
        with tempfile.TemporaryDirectory() as td:
            inp = os.path.join(td, "in.pkl")
            outp = os.path.join(td, "out.pkl")
            with open(inp, "wb") as f:
                pickle.dump(in_maps, f)
            code = (f"import sys; sys.path.insert(0, {here!r}); "
                    f"import kernel; kernel._subproc_main({inp!r}, {outp!r})")
            try:
                r = subprocess.run([sys.executable, "-c", code], timeout=1800)
                if r.returncode == 0 and os.path.exists(outp):
                    with open(outp, "rb") as f:
                        return pickle.load(f)
                last = RuntimeError(f"subprocess rc={r.returncode}")
            except Exception as e:  # noqa: BLE001
                last = e
    raise RuntimeError(f"device execution failed after retries: {last}")


def kernel(**inputs) -> np.ndarray:
    key = ("prog", X_DT)
    if key not in _CACHE:
        _CACHE[key] = _build_program(X_DT)
    nc = _CACHE[key]

    in_maps, units = _prep(inputs)
    try:
        results = _run_device(nc, in_maps)
    except Exception:
        # A crashed NEFF execution can poison this process's jax runtime
        # (NRT_EXEC_UNIT_UNRECOVERABLE); a fresh process recovers reliably.
        results = _run_via_subprocess(in_maps)

    x_loc = np.zeros((B, D, HW, HW), np.float32)
    for core in range(N_CORES):
        xl = results[core]["xloc"]  # [128 pix, 256 c]
        for u, (b, i) in enumerate(units[2 * core:2 * core + 2]):
            blk = xl[64 * u:64 * u + 64, :].reshape(4, 4, 4, D).transpose(3, 1, 0, 2)
            x_loc[b, :, 4 * i:4 * i + 4, :] = blk.reshape(D, 4, 16)
    return np.concatenate([np.asarray(inputs["x"], np.float32), x_loc], axis=1)


# revision 22
# speedup vs baseline: 1.0244x; 1.0128x over previous
"""Trainium2 Bass kernel for nn_Block_Attention_3 (sparse_attention).

Contract: kernel(**inputs) takes FULL fp32 inputs (as in reference.setup_inputs())
and returns the FULL (4, 2304, 16, 16) fp32 output.

Strategy (zero-collective position sharding + fp8 weight streaming):
  The image is 16x16 = 4x4 grid of 4x4 patches. All cross-position coupling in
  the block stays within one (batch, patch-row) group, so the 16 units (b, i)
  shard cleanly across 8 cores, 2 units/core, with weights replicated.
  Replication is the whole cost: the kernel is DMA-stream-bound on the four
  2048x256 conv weight matrices. They are streamed as float8e3 (e3m4) with
  per-tensor scales, halving weight traffic vs bf16; x streams as e3m4 too.
  Every descale folds into host-prepared operands (R, pos copies, bias rows)
  or rides per-partition scale columns, so the compiled program is
  input-independent and exact up to the fp8 rounding itself.

Per-core pipeline (single Bass program, SPMD over 8 cores):
  - inference BN folded into conv weights/biases on host; out-BN scale into the
    V path; v-bias + k-bias ride the pos operands; bd/bq/beta enter as rank-1
    matmuls at the END of each PSUM accumulation group.
  - pixels are laid out patch-major: pix = u*64 + 16*jp + 4*ph + pw.
  - d,v convs accumulate in A-layout [pix, outch]; k,q convs in B-layout
    [outch, pix] so the scores matmul needs no transposes.
  - big tensors stream via SP-queue HWDGE (9 DMAs, sized so desc-gen stays
    ahead of the 360 B/ns transfer stream); small aux tensors go via Pool
    SWDGE, off the shared HWDGE device entirely.
  - a chain of wide dummy matmuls from t~0.5us keeps the PE p-state ramp
    warm so the real convs run at full clock (0.42 ns/row, not 0.83).
  - attention for all 8 patches runs as one batched 128x128 matmul pair; the
    block-diagonal -30000 mask is pre-accumulated into the scores PSUM via a
    single K=9 matmul; the V tail is split so only a 4-chunk conv, one matmul
    pair and one small DMA sit after the last weight byte lands.
"""
import os
import sys

sys.path.insert(0, "/opt/trn_rl_repo")

import numpy as np

EPS = 1e-5
D_IN, D, B, HW, P = 2048, 256, 4, 16, 4
NCHUNK = D_IN // 128  # 16
N_CORES = 8
MASK_NEG = 30000.0
F8MAX = 15.5  # TRN float8e3 (e3m4) max normal
X_DT = os.environ.get("KERNEL_XDT", "float8e3")  # bfloat16 | float8e3
N_WARM = int(os.environ.get("KERNEL_WARM", "13"))

_CACHE = {}

# rows aux layout (bf16): [1, 896]
_R_ONES = slice(0, 128)
_R_BD = slice(128, 384)            # d-conv bias (BN+scale folded)
_R_BETA = slice(384, 640)          # out-BN beta (scale folded)
_R_BQ = slice(640, 896)            # bq0|bq1 rows [1,128] each
ROWS_LEN = 896
# aux bf16 [128, 896]: posA_sv | posb_k | posb_j | identity
# combo f32 [128, 16]: R/(sx^2 sq sk) [0:8], 1/(sx sd) [8], 1/(sx sv) [9]


def _build_program(x_dt_name: str):
    """Build (and compile to BIR) the single-core SPMD Bass program."""
    import concourse.mybir as mybir
    import concourse.tile as tile
    from concourse import bacc

    xdt = getattr(mybir.dt, x_dt_name)
    wdt = mybir.dt.float8e3
    bdt = mybir.dt.bfloat16
    f32 = mybir.dt.float32

    nc = bacc.Bacc("TRN2", target_bir_lowering=False, debug=False,
                   num_devices=N_CORES)

    xcols = NCHUNK * 128
    x_d = nc.dram_tensor("x", [128, xcols], xdt, kind="ExternalInput")
    w_d = {name: nc.dram_tensor(name, [128, NCHUNK * 256], wdt,
                                kind="ExternalInput")
           for name in ("wd", "wv", "wk", "wq")}
    aux_d = nc.dram_tensor("aux", [128, 896], bdt, kind="ExternalInput")
    combo_d = nc.dram_tensor("combo", [128, 16], f32, kind="ExternalInput")
    rows_d = nc.dram_tensor("rows", [1, ROWS_LEN], bdt, kind="ExternalInput")
    mask9_d = nc.dram_tensor("mask9", [9, 256], bdt, kind="ExternalInput")
    out_d = nc.dram_tensor("xloc", [128, 256], f32, kind="ExternalOutput")

    with tile.TileContext(nc) as tc:
        with (
            tc.tile_pool(name="big", bufs=1) as big,
            tc.tile_pool(name="small", bufs=1) as small,
            tc.tile_pool(name="ps", bufs=1, space="PSUM") as ps,
            tc.tile_pool(name="ps2", bufs=2, space="PSUM") as ps2,
        ):
            xt = big.tile([128, xcols], xdt, tag="xt")
            wt = {n: big.tile([128, NCHUNK * 256], wdt, tag=n, name=n + "_t")
                  for n in ("wd", "wv", "wk", "wq")}
            aux = small.tile([128, 896], bdt, tag="aux")
            combo = small.tile([128, 16], f32, tag="combo")
            rows = small.tile([1, ROWS_LEN], bdt, tag="rows")
            mask9 = small.tile([9, 256], bdt, tag="mask9")

            # ---- PE p-state warmup: wide dummy matmuls back-to-back from
            # ~0.5us keep the tensor clock ramping while the stream lands ----
            if N_WARM:
                wsrc = small.tile([1, 256], bdt, tag="wsrc")
                nc.gpsimd.memset(wsrc[:], 0.0)
                warm_ps = ps2.tile([1, 256], f32, tag="post", name="warm_ps")
                for _ in range(N_WARM):
                    nc.tensor.matmul(warm_ps[:], wsrc[0:1, 0:1], wsrc[:],
                                     start=True, stop=True)

            # ---- DMA loads. Big tensors: SP HWDGE, compute order; x/wd
            # halves let the d conv start while the rest streams.
            # Small aux tensors: Pool SWDGE (no shared-HWDGE contention). ----
            # Pool SWDGE pipe: x half 1 (lands ~3.7us, well before the
            # d-conv tail needs it) + small latency-tolerant tensors.
            xh = xcols // 2
            nc.gpsimd.dma_start(xt[:, xh:xcols], x_d.ap()[:, xh:xcols])
            nc.gpsimd.dma_start(rows[:], rows_d.ap())
            nc.gpsimd.dma_start(combo[:], combo_d.ap())
            nc.gpsimd.dma_start(mask9[:], mask9_d.ap())

            nc.sync.dma_start(wt["wd"][:, 0:2048], w_d["wd"].ap()[:, 0:2048])
            nc.sync.dma_start(xt[:, 0:xh], x_d.ap()[:, 0:xh])
            nc.sync.dma_start(wt["wd"][:, 2048:4096],
                              w_d["wd"].ap()[:, 2048:4096])
            nc.sync.dma_start(wt["wq"][:, 0:2048], w_d["wq"].ap()[:, 0:2048])
            nc.sync.dma_start(wt["wq"][:, 2048:4096],
                              w_d["wq"].ap()[:, 2048:4096])
            nc.sync.dma_start(aux[:], aux_d.ap())
            nc.sync.dma_start(wt["wk"][:, 0:2048], w_d["wk"].ap()[:, 0:2048])
            nc.sync.dma_start(wt["wk"][:, 2048:4096],
                              w_d["wk"].ap()[:, 2048:4096])
            nc.sync.dma_start(wt["wv"][:, 0:2048], w_d["wv"].ap()[:, 0:2048])
            nc.sync.dma_start(wt["wv"][:, 2048:3584],
                              w_d["wv"].ap()[:, 2048:3584])
            nc.sync.dma_start(wt["wv"][:, 3584:4096],
                              w_d["wv"].ap()[:, 3584:4096])

            posa = aux[:, 0:256]
            posbk = aux[:, 256:512]
            posbj = aux[:, 512:768]
            ident = aux[:, 768:896]
            R_ap = combo[:, 0:8]
            dscale = combo[:, 8:9]
            oscale = combo[:, 9:10]
            ones_r = rows[0:1, _R_ONES]

            # ---- conv PSUM accumulators ----
            d_ps = ps2.tile([128, 256], f32, tag="post", name="d_ps")
            kq_ps = [[ps.tile([128, 128], f32, tag=f"{n}{h}_ps", name=f"{n}{h}_ps")
                      for h in range(2)] for n in ("k", "q")]
            v_ps = [ps.tile([128, 128], f32, tag=f"v{g}_ps", name=f"v{g}_ps")
                    for g in range(2)]

            def a_conv_d(c0, c1):
                # A-layout [pix, outch], x chunk stationary
                for c in range(c0, c1):
                    nc.tensor.matmul(d_ps[:], xt[:, c * 128:(c + 1) * 128],
                                     wt["wd"][:, c * 256:(c + 1) * 256],
                                     start=(c == 0), stop=False)

            def b_conv(name, wi, h, brow, pre=None):
                # B-layout [outch, pix], weight chunk stationary; wk/wq packed
                # outch-half-major: col = h*2048 + c*128 + o. `pre` seeds the
                # PSUM group with a full [128,128] operand via the identity
                # rank-128 matmul; `brow` adds a rank-1 bias at the end.
                acc = kq_ps[wi][h]
                if pre is not None:
                    nc.tensor.matmul(acc[:], ident, pre, start=True, stop=False)
                for c in range(NCHUNK):
                    nc.tensor.matmul(
                        acc[:],
                        wt[name][:, h * 2048 + c * 128:h * 2048 + (c + 1) * 128],
                        xt[:, c * 128:(c + 1) * 128],
                        start=(pre is None and c == 0),
                        stop=(brow is None and c == NCHUNK - 1))
                if brow is not None:
                    nc.tensor.matmul(acc[:], brow, ones_r, start=False, stop=True)

            def v_conv(g, c0, c1):
                # A-layout half [pix, 128], wv packed outch-half-major
                for c in range(c0, c1):
                    nc.tensor.matmul(
                        v_ps[g][:], xt[:, c * 128:(c + 1) * 128],
                        wt["wv"][:, g * 2048 + c * 128:g * 2048 + (c + 1) * 128],
                        start=(c == 0), stop=(c == NCHUNK - 1))

            AF = mybir.ActivationFunctionType

            # ---- PE: d conv (chunks track the x/wd half DMAs), bias last ----
            a_conv_d(0, 8)
            a_conv_d(8, NCHUNK)
            nc.tensor.matmul(d_ps[:], ones_r, rows[0:1, _R_BD],
                             start=False, stop=True)

            # ---- d path: relu -> exp(scale*in, accum); normalization folds
            # into R (S = einc.T @ (R*dsuminv)), skipping the incx tile ----
            inc = small.tile([128, 256], f32, tag="inc")
            nc.scalar.activation(inc[:], d_ps[:], AF.Relu)
            einc = small.tile([128, 256], f32, tag="einc")
            dsum = small.tile([128, 1], f32, tag="dsum")
            nc.scalar.activation(einc[:], inc[:], AF.Exp, scale=dscale,
                                 accum_out=dsum[:])
            dsuminv = small.tile([128, 1], f32, tag="dsuminv")
            nc.vector.reciprocal(dsuminv[:], dsum[:])
            sR = small.tile([128, 8], f32, tag="sR")
            nc.vector.tensor_scalar_mul(sR[:], R_ap, dsuminv[:, 0:1])
            s_ps = ps2.tile([128, 16], f32, tag="post", name="s_ps")
            sT = small.tile([128, 16], f32, tag="sT")
            for h in range(2):
                nc.tensor.matmul(s_ps[:, h * 8:(h + 1) * 8],
                                 einc[:, h * 128:(h + 1) * 128],
                                 sR[:], start=True, stop=True,
                                 skip_group_check=(h == 1))
            nc.vector.tensor_copy(sT[:], s_ps[:])

            # ---- scores PSUM: block mask first (one K=9 matmul) ----
            sc_ps = ps2.tile([128, 128], f32, tag="post", name="sc_ps")
            nc.tensor.matmul(sc_ps[:], mask9[:, 0:128], mask9[:, 128:256],
                             start=True, stop=False)

            # ---- q convs (B-layout); jtmp = q_psum * S broadcast ----
            jtmp = [small.tile([128, 128], bdt, tag=f"jtmp{h}", name=f"jtmp{h}")
                    for h in range(2)]
            for h in range(2):
                b_conv("wq", 1, h, rows[0:1, 640 + h * 128:768 + h * 128])
                s_bcast = sT[:, h * 8:(h + 1) * 8].unsqueeze(2).broadcast_to((128, 8, 16))
                q3d = kq_ps[1][h][:].rearrange("p (b w) -> p b w", b=8)
                j3d = jtmp[h][:].rearrange("p (b w) -> p b w", b=8)
                nc.vector.tensor_tensor(j3d, q3d, s_bcast, op=mybir.AluOpType.mult)

            # ---- k convs: posbk (incl bk) seeds the PSUM group, so Kp is a
            # plain PSUM->SBUF copy on the scalar engine ----
            kp = [small.tile([128, 128], bdt, tag=f"kp{h}", name=f"kp{h}")
                  for h in range(2)]
            for h in range(2):
                b_conv("wk", 0, h, None, pre=posbk[:, h * 128:(h + 1) * 128])
                nc.scalar.activation(kp[h][:], kq_ps[0][h][:], AF.Copy)

            # ---- scores + v convs interleaved on PE. The posbj term rides
            # the scores PSUM as Kp.T @ posbj, so no vector add for J. ----
            nc.tensor.matmul(sc_ps[:], kp[0][:], posbj[:, 0:128],
                             start=False, stop=False)
            nc.tensor.matmul(sc_ps[:], kp[0][:], jtmp[0][:],
                             start=False, stop=False)
            v_conv(0, 0, 10)
            nc.tensor.matmul(sc_ps[:], kp[1][:], posbj[:, 128:256],
                             start=False, stop=False)
            nc.tensor.matmul(sc_ps[:], kp[1][:], jtmp[1][:],
                             start=False, stop=True)
            v_conv(0, 10, NCHUNK)
            v_conv(1, 0, 12)
            v_conv(1, 12, NCHUNK)

            nmx = small.tile([128, 1], f32, tag="nmx")
            nc.vector.reduce_max(nmx[:], sc_ps[:], axis=mybir.AxisListType.X,
                                 negate=True)
            e_t = small.tile([128, 128], f32, tag="e_t")
            den = small.tile([128, 1], f32, tag="den")
            nc.scalar.activation(e_t[:], sc_ps[:], AF.Exp, bias=nmx[:, 0:1],
                                 accum_out=den[:])
            vpt = small.tile([128, 256], bdt, tag="vpt")
            xloc = small.tile([128, 256], f32, tag="xloc")
            nc.vector.tensor_tensor(vpt[:, 0:128], v_ps[0][:], posa[:, 0:128],
                                    op=mybir.AluOpType.add)
            deninv = small.tile([128, 1], f32, tag="deninv")
            nc.vector.reciprocal(deninv[:], den[:])
            att = small.tile([128, 128], bdt, tag="att")
            nc.vector.tensor_scalar_mul(att[:], e_t[:], deninv[:, 0:1])
            nc.vector.tensor_tensor(vpt[:, 128:256], v_ps[1][:],
                                    posa[:, 128:256], op=mybir.AluOpType.add)
            att_ps = [ps.tile([128, 128], f32, tag=f"q{g}_ps",
                              name=f"att_ps{g}") for g in range(2)]
            for g in range(2):
                gs = slice(g * 128, (g + 1) * 128)
                nc.tensor.matmul(att_ps[g][:], ones_r,
                                 rows[0:1, 384 + g * 128:384 + (g + 1) * 128],
                                 start=True, stop=False)
                nc.tensor.matmul(att_ps[g][:], att[:], vpt[:, gs], start=False,
                                 stop=True)
            nc.scalar.activation(xloc[:, 0:128], att_ps[0][:], AF.Copy,
                                 scale=oscale)
            nc.vector.tensor_scalar_mul(xloc[:, 128:256], att_ps[1][:], oscale)
            nc.sync.dma_start(out_d.ap()[:], xloc[:])

    nc.compile()
    return nc


def _fold_bn(w, b, g, beta, m, v):
    s = g / np.sqrt(v + EPS)
    return (w * s[:, None]).astype(np.float32), (s * (b - m) + beta).astype(np.float32)


def _np_dt(name):
    import ml_dtypes
    if name == "bfloat16":
        return ml_dtypes.bfloat16
    if name == "float8e3":
        return ml_dtypes.float8_e3m4
    return np.float32


def _prep(inputs):
    """Host-side prep: BN folds, fp8 quantization + per-core input maps."""
    import ml_dtypes
    bf = ml_dtypes.bfloat16
    f8 = ml_dtypes.float8_e3m4
    x_np_dt = _np_dt(X_DT)

    inp = {k: np.asarray(v, dtype=np.float32) for k, v in inputs.items()}
    x, pos = inp["x"], inp["pos"]
    wk, bk = _fold_bn(inp["wk"], inp["bk"], inp["gk"], inp["betak"], inp["mk"], inp["vk"])
    wq, bq = _fold_bn(inp["wq"], inp["bq"], inp["gq"], inp["betaq"], inp["mq"], inp["vq"])
    wv, bv = _fold_bn(inp["wv"], inp["bv"], inp["gv"], inp["betav"], inp["mv"], inp["vv"])
    wd, bd = _fold_bn(inp["wd"], inp["bd"], inp["gd"], inp["betad"], inp["md"], inp["vd"])
    so = (inp["go"] / np.sqrt(inp["vo"] + EPS)).astype(np.float32)
    beta_o = (inp["beto"] - inp["mo"] * so).astype(np.float32)
    wv = wv * so[:, None]
    bv = bv * so

    # per-tensor fp8 scales; s_x = 1 when x stays bf16
    s_x = float(F8MAX / np.abs(x).max()) if X_DT == "float8e3" else 1.0
    sc = {n: float(F8MAX / np.abs(w).max())
          for n, w in (("d", wd), ("k", wk), ("q", wq), ("v", wv))}

    def quant(w, s):
        return np.clip(w * s, -F8MAX, F8MAX).astype(f8)

    def wpack_cmaj(w):  # chunk-major: [256 out, 2048 in] -> [128, (c,256)]
        wt = w.T.reshape(NCHUNK, 128, 256).transpose(1, 0, 2).reshape(128, -1)
        return np.ascontiguousarray(wt)

    def wpack_hmaj(w):  # outch-half-major: [256 out, 2048 in] -> [128,(h,c,128)]
        wt = w.T.reshape(NCHUNK, 128, 2, 128).transpose(1, 2, 0, 3).reshape(128, -1)
        return np.ascontiguousarray(wt)

    w_packed = {"wd": wpack_cmaj(quant(wd, sc["d"])),
                "wv": wpack_hmaj(quant(wv, sc["v"])),
                "wk": wpack_hmaj(quant(wk, sc["k"])),
                "wq": wpack_hmaj(quant(wq, sc["q"]))}

    p_idx = np.arange(128)
    R = np.zeros((128, 8), np.float32)
    R[p_idx, (p_idx // 64) * 4 + (p_idx % 16) // 4] = 1.0
    R /= (s_x * s_x * sc["q"] * sc["k"])
    pix_patch = (p_idx // 64) * 4 + (p_idx % 64) // 16
    blk_ind = (pix_patch[None, :] == np.arange(8)[:, None]).astype(np.float32)

    rows = np.zeros((1, ROWS_LEN), np.float32)
    rows[0, _R_ONES] = 1.0
    rows[0, _R_BD] = bd * (s_x * sc["d"])
    rows[0, _R_BETA] = beta_o * (s_x * sc["v"])
    rows[0, _R_BQ] = bq * (s_x * sc["q"])
    rows = rows.astype(bf)

    mask9 = np.zeros((9, 256), np.float32)
    mask9[0, 0:128] = 1.0
    mask9[0, 128:256] = -MASK_NEG
    mask9[1:9, 0:128] = blk_ind
    mask9[1:9, 128:256] = blk_ind * MASK_NEG
    mask9 = mask9.astype(bf)

    units = [(b, i) for b in range(B) for i in range(P)]
    in_maps = []
    for core in range(N_CORES):
        cu = units[2 * core:2 * core + 2]
        x_sb = np.empty((128, NCHUNK, 128), np.float32)
        pos_A = np.empty((128, 256), np.float32)
        posb_sb = np.empty((128, 256), np.float32)
        for u, (b, i) in enumerate(cu):
            # [c, ph, jp, pw] -> patch-major pixel (jp, ph, pw)
            xs = x[b, :, 4 * i:4 * i + 4, :].reshape(D_IN, 4, 4, 4)
            xs = xs.transpose(0, 2, 1, 3).reshape(D_IN, 64)
            x_sb[:, :, 64 * u:64 * u + 64] = xs.reshape(NCHUNK, 128, 64).transpose(1, 0, 2)
            ps_ = pos[b, :, 4 * i:4 * i + 4, :].reshape(D, 4, 4, 4).transpose(0, 2, 1, 3).reshape(D, 64)
            pos_A[64 * u:64 * u + 64, :] = ps_.T
            posb_sb[:, 64 * u:64 * u + 64] = ps_[0:128]
            posb_sb[:, 128 + 64 * u:128 + 64 * u + 64] = ps_[128:256]
        pos_A_sov = (pos_A * so[None, :] + bv[None, :]) * (s_x * sc["v"])
        # fold bk (per out-channel) into posbk: posb layout [ch, h*128+pix]
        posb_k = posb_sb.copy()
        for h in range(2):
            posb_k[:, h * 128:(h + 1) * 128] += bk[h * 128:(h + 1) * 128][:, None]
        posb_k = posb_k * (s_x * sc["k"])
        posb_j = posb_sb / (s_x * sc["k"])
        aux = np.concatenate([pos_A_sov, posb_k, posb_j,
                              np.eye(128, dtype=np.float32)], axis=1).astype(bf)
        combo = np.zeros((128, 16), np.float32)
        combo[:, 0:8] = R
        combo[:, 8] = 1.0 / (s_x * sc["d"])
        combo[:, 9] = 1.0 / (s_x * sc["v"])
        if X_DT == "float8e3":
            x_core = np.clip(x_sb.reshape(128, -1) * s_x,
                             -F8MAX, F8MAX).astype(f8)
        else:
            x_core = x_sb.reshape(128, -1).astype(bf)
        in_maps.append({
            "x": np.ascontiguousarray(x_core),
            "wd": w_packed["wd"], "wv": w_packed["wv"],
            "wk": w_packed["wk"], "wq": w_packed["wq"],
            "aux": np.ascontiguousarray(aux),
            "combo": combo, "rows": rows, "mask9": mask9,
        })
    return in_maps, units


def _run_device(nc, in_maps):
    from concourse.bass_utils import run_bass_kernel_spmd
    return run_bass_kernel_spmd(nc, in_maps, list(range(N_CORES))).results


def _subproc_main(inp_path, out_path):
    import pickle
    with open(inp_path, "rb") as f:
        in_maps = pickle.load(f)
    nc = _build_program(X_DT)
    res = _run_device(nc, in_maps)
    with open(out_path, "wb") as f:
        pickle.dump(res, f)


def _run_via_subprocess(in_maps):
    import pickle
    import subprocess
    import tempfile
    here = os.path.dirname(os.path.abspath(__file__))
    last = None
    for _ in range(2):
        with tempfile.TemporaryDirectory() as td:
            inp = os.path.join(td, "in.pkl")
            outp = os.path.join(td, "out.pkl")
            with open(inp, "wb") as f:
                pickle.dump(in_maps, f)
            code = (f"import sys; sys.path.insert(0, {here!r}); "
                    f"import kernel; kernel._subproc_main({inp!r}, {outp!r})")
            try:
                r = subprocess.run([sys.executable, "-c", code], timeout=1800)
                if r.returncode == 0 and os.path.exists(outp):
                    with open(outp, "rb") as f:
                        return pickle.load(f)
                last = RuntimeError(f"subprocess rc={r.returncode}")
            except Exception as e:  # noqa: BLE001
                last = e
    raise RuntimeError(f"device execution failed after retries: {last}")


def kernel(**inputs) -> np.ndarray:
    key = ("prog", X_DT)
    if key not in _CACHE:
        _CACHE[key] = _build_program(X_DT)
    nc = _CACHE[key]

    in_maps, units = _prep(inputs)
    try:
        results = _run_device(nc, in_maps)
    except Exception:
        # A crashed NEFF execution can poison this process's jax runtime
        # (NRT_EXEC_UNIT_UNRECOVERABLE); a fresh process recovers reliably.
        results = _run_via_subprocess(in_maps)

    x_loc = np.zeros((B, D, HW, HW), np.float32)
    for core in range(N_CORES):
        xl = results[core]["xloc"]  # [128 pix, 256 c]
        for u, (b, i) in enumerate(units[2 * core:2 * core + 2]):
            blk = xl[64 * u:64 * u + 64, :].reshape(4, 4, 4, D).transpose(3, 1, 0, 2)
            x_loc[b, :, 4 * i:4 * i + 4, :] = blk.reshape(D, 4, 16)
    return np.concatenate([np.asarray(inputs["x"], np.float32), x_loc], axis=1)
